# revision 23
# baseline (speedup 1.0000x reference)
"""2-layer GAT + mean-pool + MLP on 8 Trainium2 NeuronCores (Bass/Tile).

v2 strategy (self-contained; shapes hardcoded for the nn_GAT problem):
  - dst nodes split into 4 quarters; each quarter handled by a core PAIR
    (2q, 2q+1). Within a pair, edges are split by src-parity so each
    core's gather table has < 32768 rows (dma_gather uses int16 indices).
  - Feature tables hold fp8 rows (512B stride, 272B gathered payload:
    [256B feat fp8 | 16B el bf16]) to minimize the serialized per-edge
    gather traffic; the escore multiply reads fp8 directly on DVE.
  - Layer-0 table is built fully LOCALLY from host-transposed replicated
    h (hTb input: all parity-p nodes in table-row order) -- no
    collective, no barrier. er for all 98 windows of the core's quarter
    is computed locally (hTo = own half, hTq = partner half) and written
    via two parity-offset indirect scatters into a LOCAL 256B-stride er
    table for 32B-payload per-edge er gathers.
  - Layer-1 table shards (built during layer-0 phase B) are exchanged
    with one parity-group AllGather; er2 goes through pair-shared HBM
    with one pair barrier.
  - Main loop: per GRP-window group, dma_gather edge rows by src (544B)
    and er rows by dst (32B); escore = max(exp(el+er), exp(0.2(el+er)))
    == exp(leaky_relu(el+er)); rhs = [feat*escore | escore]; per
    128-dst window, one-hot S via is_equal against iota; PE matmul
    psum[dst, 0:264] += S.T @ rhs accumulates messages AND softmax
    denominators.
  - Phase A computes partner-half windows, ships partials through
    pair-shared HBM (indirect DMA + tiny pair AllGather barrier);
    phase B merges and normalizes own-half windows.
  - Per-graph mean-pool via one-hot matmul accumulation, AllReduce of
    the [64, 257] partials (sums + counts), then a small f32 MLP.
"""
import numpy as np
import ml_dtypes

P = 128
HEADS, HID = 8, 32
D = HEADS * HID          # 256
IN_DIM = 128
NCLS = 10
NEG = 0.2
ROWB = 512               # feat-table row stride bytes
GROWB = 272              # gathered feat payload bytes [256 fp8 feat|16 el bf16]
ERB = 256                # er-table row stride bytes
GERB = 16                # gathered er payload bytes (8 x bf16)
COLS_OUT = D + HEADS     # 264: [msg | escore]
GRP = 4                  # window slots per gather group


class Cfg:
    def __init__(self, N, E, G):
        self.N, self.E, self.G = N, E, G
        assert N % 4 == 0
        self.QUARTER = N // 4
        self.WIN = -(-self.QUARTER // P)
        assert self.WIN % 2 == 0
        self.WPH = self.WIN // 2
        self.HALF = self.WPH * P              # rows per parity half-quarter
        self.TROWS = 4 * self.HALF            # per-parity table rows
        assert self.TROWS < 32768
        self.ER_ROWS = P * self.WIN


FULL = Cfg(50000, 800000, 64)


def _bf16(x):
    return np.asarray(x, np.float32).astype(ml_dtypes.bfloat16)


def _pack_idx16(vals):
    """vals: [C, 128] int -> dma_gather idx layout [128, C*8] int16."""
    C = vals.shape[0]
    a = vals.reshape(C, 8, 16).transpose(2, 0, 1).reshape(16, C * 8)
    return np.tile(a.astype(np.int16), (8, 1))


def host_prep(cfg, h, src, dst, graph_ids, W1, al1, ar1, b1, W2, al2, ar2, b2,
              Wc1, bc1, Wc2, bc2, Wc3, bc3):
    """Build per-core input maps + compile-time chunk structure."""
    N, E, G = cfg.N, cfg.E, cfg.G
    Q, WIN, WPH, HALF = cfg.QUARTER, cfg.WIN, cfg.WPH, cfg.HALF

    src = np.asarray(src, np.int64)
    dst = np.asarray(dst, np.int64)
    graph_ids = np.asarray(graph_ids, np.int64)
    h = np.asarray(h, np.float32)

    q = dst // Q
    o = dst - q * Q
    w = o >> 7
    dl = o & 127
    so = src % Q
    sq = src // Q
    spar = (so >= HALF).astype(np.int64)
    srow = sq * HALF + (so - HALF * spar)
    er_idx = dl * WIN + w
    core = 2 * q + spar

    key = core * WIN + w
    order = np.lexsort((srow, key))
    key_s = key[order]
    srow_s = srow[order]
    er_s = er_idx[order]
    dl_s = dl[order]
    counts = np.bincount(key_s, minlength=8 * WIN).reshape(8, WIN)
    starts = np.zeros(8 * WIN + 1, np.int64)
    np.cumsum(counts.reshape(-1), out=starts[1:])

    # slot -> window mapping per parity: phase A slot i -> win i + WPH*(1-p);
    # phase B slot j -> win j + WPH*p
    def slot_win(p, s):
        if s < WPH:
            return s + WPH * (1 - p)
        return (s - WPH) + WPH * p

    cpw = []
    for s in range(WIN):
        m = 1
        for c in range(8):
            n = counts[c, slot_win(c & 1, s)]
            m = max(m, -(-int(n) // P))
        cpw.append(m)
    C = sum(cpw)
    offs = np.zeros(WIN + 1, np.int64)
    np.cumsum(cpw, out=offs[1:])

    in_maps = []
    HROWS_MAX = 3 * Q + 2 * HALF
    h_pad = np.zeros((HROWS_MAX, IN_DIM), np.float32)
    h_pad[:N] = h
    al_bd = np.zeros((D, HEADS), np.float32)
    ar_bd = np.zeros((D, HEADS), np.float32)
    for hh in range(HEADS):
        al_bd[hh * HID:(hh + 1) * HID, hh] = np.asarray(al1, np.float32)[hh]
        ar_bd[hh * HID:(hh + 1) * HID, hh] = np.asarray(ar1, np.float32)[hh]
    Welr1 = np.concatenate(
        [np.asarray(W1, np.float32) @ al_bd, np.asarray(W1, np.float32) @ ar_bd],
        axis=1)  # [IN_DIM, 16]
    al_bd2 = np.zeros((D, HEADS), np.float32)
    ar_bd2 = np.zeros((D, HEADS), np.float32)
    for hh in range(HEADS):
        al_bd2[hh * HID:(hh + 1) * HID, hh] = np.asarray(al2, np.float32)[hh]
        ar_bd2[hh * HID:(hh + 1) * HID, hh] = np.asarray(ar2, np.float32)[hh]
    Welr2 = np.concatenate(
        [np.asarray(W2, np.float32) @ al_bd2, np.asarray(W2, np.float32) @ ar_bd2],
        axis=1)  # [D, 16]

    # feature columns head-minor (newcol j*8+h = oldcol h*32+j): the per-head
    # escore broadcast becomes a stride-1 inner dim on DVE (2x mode, no
    # materialized broadcast needed)
    perm = np.arange(D).reshape(HEADS, HID).T.reshape(-1)
    shared = {
        "W1b": _bf16(W1).reshape(IN_DIM, D)[:, perm],
        "W2b": _bf16(W2).reshape(D, D)[perm][:, perm],
        "Welr1b": _bf16(Welr1),
        "Welr2b": _bf16(np.asarray(Welr2, np.float32)[perm, :]),
        "b1rep": np.tile(np.asarray(b1, np.float32)[perm][None, :], (P, 1)),
        "b2rep": np.tile(np.asarray(b2, np.float32)[perm][None, :], (P, 1)),
        "Wc1": np.asarray(Wc1, np.float32)[perm, :],
        "Wc2": np.asarray(Wc2, np.float32),
        "Wc3": np.asarray(Wc3, np.float32),
        "bc1rep": np.tile(np.asarray(bc1, np.float32)[None, :], (64, 1)),
        "bc2rep": np.tile(np.asarray(bc2, np.float32)[None, :], (64, 1)),
        "bc3rep": np.tile(np.asarray(bc3, np.float32)[None, :], (64, 1)),
        "identity": np.eye(P, dtype=np.float32),
        "iota_bf": np.tile(np.arange(P, dtype=np.float32)[None, :],
                           (P, 1)).astype(ml_dtypes.bfloat16),
        "ones_bf": np.ones((P, 1), ml_dtypes.bfloat16),
    }

    for c in range(8):
        qq, p = c // 2, c & 1
        i1 = np.zeros((C, P), np.int64)
        i2 = np.zeros((C, P), np.int64)
        dlv = np.full((C, P), -1.0, np.float32)
        for s in range(WIN):
            wv = slot_win(p, s)
            a, b_ = starts[c * WIN + wv], starts[c * WIN + wv + 1]
            n = b_ - a
            base = offs[s]
            nchunk = cpw[s]
            buf1 = np.zeros(nchunk * P, np.int64)
            buf2 = np.zeros(nchunk * P, np.int64)
            bufd = np.full(nchunk * P, -1.0, np.float32)
            buf1[:n] = srow_s[a:b_]
            buf2[:n] = er_s[a:b_]
            bufd[:n] = dl_s[a:b_]
            # slot j -> (partition j%128, chunk j//128)
            i1[base:base + nchunk] = buf1.reshape(nchunk, P)
            i2[base:base + nchunk] = buf2.reshape(nchunk, P)
            dlv[base:base + nchunk] = bufd.reshape(nchunk, P)

        base_node = qq * Q + p * HALF
        gl = np.full((P, WPH), -1.0, np.float32)
        for j in range(WPH):
            wv = j + WPH * p
            nd = base_node + j * P + np.arange(P)
            valid = (wv * P + np.arange(P) < Q) & (nd < N)
            gl[valid, j] = graph_ids[nd[valid]]

        # hTb: h^T for all parity-p nodes, table-row order (r = qq2*HALF+off)
        rows = np.concatenate([
            np.arange(HALF) + qq2 * Q + p * HALF for qq2 in range(4)])
        hTb = _bf16(h_pad[rows].T)                         # [128, TROWS]
        # hTo / hTq: h^T for own-quarter own / partner parity half
        rows_o = np.arange(HALF) + qq * Q + p * HALF
        rows_q = np.arange(HALF) + qq * Q + (1 - p) * HALF
        hTo = _bf16(h_pad[rows_o].T)                       # [128, HALF]
        hTq = _bf16(h_pad[rows_q].T)                       # [128, HALF]

        # layer-1 table row layout with split AG: windows 0..23 of each
        # shard land in region A (rows sq*3072 + off), windows 24..48 in
        # region B (rows 4*3072 + sq*3200 + (off - 3072))
        offA = HALF // 2 - 64          # 3072 = 24 windows
        i1b = np.where(i1 % HALF < offA,
                       (i1 // HALF) * offA + (i1 % HALF),
                       4 * offA + (i1 // HALF) * (HALF - offA)
                       + (i1 % HALF) - offA)
        in_maps.append(dict(
            shared,
            hTb=hTb,
            hTo=hTo,
            hTq=hTq,
            idx1=_pack_idx16(i1),
            idx1b=_pack_idx16(i1b),
            idx2=_pack_idx16(i2),
            dstl=dlv.T.copy().astype(np.float32),              # [128, C]
            gl=gl.astype(ml_dtypes.bfloat16),                   # [128, WPH]
            offs_er=(np.arange(P, dtype=np.int32) * WIN
                     + WPH * p).reshape(P, 1),
            offs_erB=(np.arange(P, dtype=np.int32) * WIN
                      + WPH * (1 - p)).reshape(P, 1),
            offs_shw=(np.arange(P, dtype=np.int32) + P * p).reshape(P, 1),
            offs_shr=(np.arange(P, dtype=np.int32) + P * (1 - p)).reshape(P, 1),
        ))
    return in_maps, cpw, offs


def build_program(cfg, cpw, offs, num_cores=8, stage=99, skip_ag=False,
                  skip_bar=False, repeat=1, omit=(), grp=GRP):
    import concourse.bass as bass
    import concourse.bacc as bacc
    import concourse.tile as tile
    import concourse.mybir as mb
    from concourse import library_config
    from concourse.tile import add_dep_helper

    WIN, WPH, HALF, TROWS, ER_ROWS = (cfg.WIN, cfg.WPH, cfg.HALF, cfg.TROWS,
                                      cfg.ER_ROWS)
    G = cfg.G
    C = sum(cpw)
    f32, bf16, u8, i16, i32 = (mb.dt.float32, mb.dt.bfloat16, mb.dt.uint8,
                               mb.dt.int16, mb.dt.int32)
    fp8 = mb.dt.float8e3
    AF = mb.ActivationFunctionType
    OP = mb.AluOpType

    nc = bacc.Bacc("TRN2", target_bir_lowering=False, debug=False,
                   enable_asserts=False, num_devices=num_cores)

    def _raw(x):
        return getattr(x, "ins", x)

    def dma_gather_raw(gp, out_ap, in_ap, idxs_ap, num_idxs, elem_size,
                       elem_step, single_packet=False, queue_num=0):
        """dma_gather (HBM source, non-transpose) without the %256
        elem_size restriction (that constraint is transpose-only in the
        Q7/decode path; stride must still be a multiple of 256B)."""
        import concourse.ap_utils as ap_utils
        from concourse._compat import exact_div
        gp._assert_queue_num(queue_num)
        assert idxs_ap.dtype == mb.dt.int16
        assert in_ap.dtype == out_ap.dtype
        assert in_ap.space.name == "DRAM"
        assert ap_utils.ap_is_contiguous(out_ap.ap[1:])
        assert ap_utils.ap_is_contiguous(idxs_ap.ap[1:])
        assert in_ap.ap[-1][1] == out_ap.ap[-1][1] == elem_size
        assert out_ap.ap[0][1] * out_ap.ap[1][1] == ((num_idxs + 127) // 128) * 128
        assert in_ap.ap[0][0] == elem_step
        stride_bytes = elem_step * mb.dt.size(in_ap.dtype)
        stride_bytes_256 = exact_div(stride_bytes, 256)
        _in_ap = gp.lower_ap_dma(in_ap, for_custom_bir_dma=True)
        inst = gp.add_instruction(
            mb.InstDMAGatherAnt(
                name=gp.bass.get_next_instruction_name(),
                ins=[*_in_ap, gp.lower_ap(idxs_ap),
                     gp.lower_val_access(gp.to_reg(num_idxs))],
                outs=[gp.lower_ap(out_ap)],
                transpose=False, num_idxs=num_idxs, elem_size=elem_size,
                stride_bytes_256=stride_bytes_256, gen_mode=0,
                single_packet=single_packet, queue_num=queue_num,
                sbuf_tokens_per_rank=0, sbuf_free_dim_per_rank=0,
                sbuf_free_dim_pad_per_rank=0, sbuf_byte_offset=0))
        return inst

    def inp(name, shape, dt):
        return nc.dram_tensor(name, list(shape), dt, kind="ExternalInput")

    hTb_in = inp("hTb", [P, TROWS], bf16)
    hTo_in = inp("hTo", [P, HALF], bf16)
    hTq_in = inp("hTq", [P, HALF], bf16)
    W1b = inp("W1b", [IN_DIM, D], bf16)
    W2b = inp("W2b", [D, D], bf16)
    Welr1b = inp("Welr1b", [IN_DIM, 16], bf16)
    Welr2b = inp("Welr2b", [D, 16], bf16)
    b1rep = inp("b1rep", [P, D], f32)
    b2rep = inp("b2rep", [P, D], f32)
    Wc1 = inp("Wc1", [D, 32], f32)
    Wc2 = inp("Wc2", [32, 32], f32)
    Wc3 = inp("Wc3", [32, NCLS], f32)
    bc1rep = inp("bc1rep", [64, 32], f32)
    bc2rep = inp("bc2rep", [64, 32], f32)
    bc3rep = inp("bc3rep", [64, NCLS], f32)
    identity = inp("identity", [P, P], f32)
    iota_in = inp("iota_bf", [P, P], bf16)
    ones_in = inp("ones_bf", [P, 1], bf16)
    idx1_in = inp("idx1", [P, C * 8], i16)
    idx1b_in = inp("idx1b", [P, C * 8], i16)
    idx2_in = inp("idx2", [P, C * 8], i16)
    dstl_in = inp("dstl", [P, C], f32)
    gl_in = inp("gl", [P, WPH], bf16)
    offs_er_in = inp("offs_er", [P, 1], i32)
    offs_erB_in = inp("offs_erB", [P, 1], i32)
    offs_shw_in = inp("offs_shw", [P, 1], i32)
    offs_shr_in = inp("offs_shr", [P, 1], i32)

    out_t = nc.dram_tensor("out", [64, NCLS], f32, kind="ExternalOutput")

    nc.gpsimd.load_library(library_config.mlp)

    with tile.TileContext(nc) as tc:
        with (
            tc.tile_pool(name="const", bufs=1) as cp,
            tc.tile_pool(name="persist", bufs=1) as pp,
            tc.tile_pool(name="work", bufs=2) as wp,
            tc.tile_pool(name="gath", bufs=2) as gpool,
            tc.tile_pool(name="gath3", bufs=2) as gpool3,
            tc.tile_pool(name="ps_pw", bufs=4, space="PSUM") as ps_pw,
            tc.tile_pool(name="ps_scr", bufs=2, space="PSUM") as ps_scr,
            tc.tile_pool(name="ps2", bufs=2, space="PSUM") as ps2,
            tc.tile_pool(name="dram", bufs=1, space="DRAM") as dp,
        ):
            # ---------- constants to SBUF ----------
            def load(t, shape, dt, name):
                s = cp.tile(list(shape), dt, name=name)
                nc.scalar.dma_start(s[:], t[:, :])
                return s

            W1b_s = load(W1b, [P, D], bf16, "W1b_s")
            Welr1b_s = load(Welr1b, [P, 16], bf16, "Welr1b_s")
            W1a_s = cp.tile([P, 272], bf16, name="W1a_s")
            nc.scalar.dma_start(W1a_s[:, 0:D], W1b[:, :])
            nc.scalar.dma_start(W1a_s[:, D:272], Welr1b[:, :])
            b1_s = load(b1rep, [P, D], f32, "b1_s")
            b2_s = load(b2rep, [P, D], f32, "b2_s")
            id_s = load(identity, [P, P], f32, "id_s")
            iota_s = load(iota_in, [P, P], bf16, "iota_s")
            ones_s = load(ones_in, [P, 1], bf16, "ones_s")
            dstl_s = load(dstl_in, [P, C], f32, "dstl_s")
            gl_s = load(gl_in, [P, WPH], bf16, "gl_s")
            offs_er_s = load(offs_er_in, [P, 1], i32, "offs_er_s")
            offs_erB_s = load(offs_erB_in, [P, 1], i32, "offs_erB_s")
            offs_shw_s = load(offs_shw_in, [P, 1], i32, "offs_shw_s")
            offs_shr_s = load(offs_shr_in, [P, 1], i32, "offs_shr_s")
            W2b_s = cp.tile([P, 2 * D], bf16, name="W2b_s")
            nc.scalar.dma_start(W2b_s[:, 0:D], W2b[0:P, :])
            nc.scalar.dma_start(W2b_s[:, D:2 * D], W2b[P:2 * P, :])
            Welr2b_s = cp.tile([P, 32], bf16, name="Welr2b_s")
            nc.scalar.dma_start(Welr2b_s[:, 0:16], Welr2b[0:P, :])
            nc.scalar.dma_start(Welr2b_s[:, 16:32], Welr2b[P:2 * P, :])

            # ---------- persistent ----------
            stash = pp.tile([P, WPH * COLS_OUT], bf16, name="stash")
            stg2 = [pp.tile([P, 2 * ROWB], u8, name=f"stg2_{i}")
                    for i in (0, 1)]
            nc.vector.memset(stg2[0][:], 0)
            nc.vector.memset(stg2[1][:], 0)
            h1T0 = pp.tile([P, HALF], bf16, name="h1T0")
            h1T1 = pp.tile([P, HALF], bf16, name="h1T1")
            elr2_st = pp.tile([P, WPH * 16], f32, name="elr2_st")
            # er_all: layer-0 uses the full [WIN] window range (own half at
            # slots [0,WPH), partner half at [WPH,WIN)); layer-1 reuses the
            # first WPH slots for er2 of own windows.
            er_all = pp.tile([P, WIN * ERB], u8, name="er_all")
            nc.vector.memset(er_all[:], 0)

            # ---------- DRAM ----------
            table = [dp.tile([TROWS, ROWB], u8, name=f"table{l}")
                     for l in (0, 1)]
            NWA = 24
            cc_in1 = dp.tile([HALF, ROWB], u8, name="cc_in1")
            er_tab0 = dp.tile([ER_ROWS, ERB], u8, name="er_tab0")
            er_tab1_all = [dp.tile([ER_ROWS, ERB], u8, name=f"er_tab1_{r}",
                                   addr_space="Shared") for r in range(repeat)]
            pship_all = [[dp.tile([2 * P, WPH * COLS_OUT], bf16,
                                  name=f"pship{l}_{r}", addr_space="Shared")
                          for l in (0, 1)] for r in range(repeat)]
            hold = {"er_tab1": er_tab1_all[0], "pship": pship_all[0]}
            bar_bufs = []
            for i in range(3 * repeat):
                bi = dp.tile([1, P], f32, name=f"bar_in{i}")
                bo = dp.tile([2, P], f32, name=f"bar_out{i}")
                bar_bufs.append((bi, bo))
            ar_in = dp.tile([64, D + 1], f32, name="ar_in")
            ar_out = dp.tile([64, D + 1], f32, name="ar_out")

            PAIRS = [[2 * i, 2 * i + 1] for i in range(num_cores // 2)]
            PARITY = ([[c for c in range(num_cores) if c % 2 == 0],
                       [c for c in range(num_cores) if c % 2 == 1]]
                      if num_cores > 1 else [[0]])
            nbar = [0]

            def pair_barrier(dep_insts, name):
                bi, bo = bar_bufs[nbar[0]]
                nbar[0] += 1
                if num_cores == 1:
                    cc = nc.scalar.dma_start(bo[0:1, :], bi[:, :])
                else:
                    cc = nc.gpsimd.collective_compute(
                        "AllGather", OP.bypass, replica_groups=PAIRS,
                        ins=[bi[:].opt()], outs=[bo[:].opt()])
                for di in dep_insts:
                    add_dep_helper(_raw(cc), _raw(di), sync=True, reason=name)
                return cc

            # =========================================================
            # layer-0: local table build (196 windows) + er for own
            # quarter's 98 windows + er-table scatters
            # =========================================================
            def table0_build():
                # 4 windows per hT load; 2-window macro-steps; feat+el in one
                # fused matmul against [W1b | Welr1b]
                for jb in range(WPH):
                    hblk = wp.tile([P, 4 * P], bf16, name="hblk", tag="hblk")
                    nc.sync.dma_start(hblk[:],
                                      hTb_in[:, jb * 4 * P:(jb + 1) * 4 * P])
                    for u in range(4):
                        j = 4 * jb + u
                        stg = stg2[j % 2]
                        pfa = ps_scr.tile([P, 272], f32, name="pfa",
                                          tag="scr", space="PSUM")
                        nc.tensor.matmul(pfa[:],
                                         lhsT=hblk[:, u * P:(u + 1) * P],
                                         rhs=W1a_s[:], start=True, stop=True)
                        stg_f8 = stg[:].bitcast(fp8)
                        nc.vector.tensor_copy(stg_f8[:, 0:D], pfa[:, 0:D])
                        stg_bf = stg[:].bitcast(bf16)
                        nc.vector.tensor_copy(stg_bf[:, 128:136],
                                              pfa[:, D:D + 8])
                        eng = nc.sync if j % 2 == 0 else nc.scalar
                        eng.dma_start(table[0][j * P:(j + 1) * P, :],
                                      stg[:, 0:ROWB])

                er_v = er_all[:].bitcast(bf16)
                for j0 in range(0, WIN, 4):
                    nwin = min(4, WIN - j0)
                    hblk = wp.tile([P, 4 * P], bf16, name="hblk2", tag="hblk2")
                    # windows [j0, j0+nwin): j < WPH from hTo, else hTq
                    na = max(0, min(WPH - j0, nwin))
                    if na > 0:
                        nc.sync.dma_start(
                            hblk[:, 0:na * P],
                            hTo_in[:, j0 * P:(j0 + na) * P])
                    if na < nwin:
                        jj0 = j0 + na - WPH
                        nc.sync.dma_start(
                            hblk[:, na * P:nwin * P],
                            hTq_in[:, jj0 * P:(jj0 + nwin - na) * P])
                    for u in range(nwin):
                        j = j0 + u
                        pelr = ps2.tile([P, 16], f32, name="pelr2",
                                        tag="pelr", space="PSUM")
                        nc.tensor.matmul(pelr[:],
                                         lhsT=hblk[:, u * P:(u + 1) * P],
                                         rhs=Welr1b_s[:], start=True,
                                         stop=True)
                        nc.vector.tensor_copy(
                            er_v[:, j * (ERB // 2):j * (ERB // 2) + 8],
                            pelr[:, 8:16])
                # own half -> rows dl*WIN + WPH*p; partner half -> +WPH*(1-p)
                nc.gpsimd.indirect_dma_start(
                    out=er_tab0[:, :], out_offset=bass.IndirectOffsetOnAxis(
                        ap=offs_er_s[:, 0:1], axis=0),
                    in_=er_all[:, 0:WPH * ERB], in_offset=None)
                nc.gpsimd.indirect_dma_start(
                    out=er_tab0[:, :], out_offset=bass.IndirectOffsetOnAxis(
                        ap=offs_erB_s[:, 0:1], axis=0),
                    in_=er_all[:, WPH * ERB:WIN * ERB], in_offset=None)

            # =========================================================
            # layer-1 table exchange (rows built during layer-0 phase B)
            # =========================================================
            ag_insts = []

            def table1_exchange_A():
                pass

            def table1_exchange():
                ersc = nc.gpsimd.indirect_dma_start(
                    out=hold["er_tab1"][:, :],
                    out_offset=bass.IndirectOffsetOnAxis(
                        ap=offs_er_s[:, 0:1], axis=0),
                    in_=er_all[:, 0:WPH * ERB], in_offset=None)
                if not skip_ag:
                    if num_cores == 1:
                        for qq in range(4):
                            nc.scalar.dma_start(
                                table[1][qq * HALF:(qq + 1) * HALF, :],
                                cc_in1[:, :])
                    else:
                        ag = nc.gpsimd.collective_compute(
                            "AllGather", OP.bypass, replica_groups=PARITY,
                            ins=[cc_in1[:].opt()], outs=[table[1][:].opt()])
                        ag_insts.append(ag)
                if skip_bar:
                    return None
                return pair_barrier([ersc], "er_bar1")

            # =========================================================
            # main loop phases for layer l
            # =========================================================
            def gather_group(l, ga, gb, er_bar):
                Bg = int(sum(cpw[ga:gb]))
                offg = int(offs[ga])
                it1 = gpool.tile([P, Bg * 8], i16, name="it1", tag="it1")
                nc.sync.dma_start(it1[:],
                                  idx1_in[:, offg * 8:(offg + Bg) * 8])
                it2 = gpool.tile([P, Bg * 8], i16, name="it2", tag="it2")
                nc.sync.dma_start(it2[:], idx2_in[:, offg * 8:(offg + Bg) * 8])
                g = gpool3.tile([P, Bg * GROWB], u8, name="g", tag="g")
                g1 = None
                if "g1" in omit:
                    nc.vector.memset(g[:, 0:4], 0)
                else:
                    g1 = dma_gather_raw(
                        nc.gpsimd,
                        out_ap=g[:].rearrange("p (b e) -> p b e", e=GROWB),
                        in_ap=bass.AP(table[l][:, :].tensor, 0,
                                      [[ROWB, TROWS], [1, GROWB]]),
                        idxs_ap=it1[:, :],
                        num_idxs=Bg * P,
                        elem_size=GROWB, elem_step=ROWB, single_packet=False)
                e = gpool3.tile([P, Bg * GERB], u8, name="e", tag="e")
                g2 = None
                if "g2" in omit:
                    nc.vector.memset(e[:, 0:4], 0)
                else:
                    ertab = er_tab0 if l == 0 else hold["er_tab1"]
                    g2 = dma_gather_raw(
                        nc.gpsimd,
                        out_ap=e[:].rearrange("p (b x) -> p b x", x=GERB),
                        in_ap=bass.AP(ertab[:, :].tensor, 0,
                                      [[ERB, ER_ROWS], [1, GERB]]),
                        idxs_ap=it2[:, :],
                        num_idxs=Bg * P,
                        elem_size=GERB, elem_step=ERB, single_packet=False)
                if er_bar is not None:
                    if g1 is not None:
                        add_dep_helper(_raw(g1), _raw(er_bar), sync=True,
                                       reason="tbar")
                        for ag in ag_insts:
                            add_dep_helper(_raw(g1), _raw(ag), sync=True,
                                           reason="agdep")
                    if g2 is not None:
                        add_dep_helper(_raw(g2), _raw(er_bar), sync=True,
                                       reason="tbar")
                return g, e, offg, Bg

            def score_group(l, g, e, offg, Bg):
                gb = g[:].bitcast(bf16).rearrange("p (b e) -> p b e",
                                                  e=GROWB // 2)
                eb = e[:].bitcast(bf16).rearrange("p (b e) -> p b e",
                                                  e=GERB // 2)
                # el sits at bf16 elems [128:136) of the gathered rows
                sc = gpool.tile([P, Bg * 8], f32, name="sc", tag="sc")
                if "sc" not in omit:
                    nc.vector.tensor_tensor(
                        out=sc[:].rearrange("p (b e) -> p b e", e=8),
                        in0=gb[:, :, 128:136], in1=eb[:, :, 0:8], op=OP.add)
                ex1 = gpool.tile([P, Bg * 8], f32, name="ex1", tag="ex1")
                ex2 = gpool.tile([P, Bg * 8], f32, name="ex2", tag="ex2")
                esc = gpool.tile([P, Bg * 8], bf16, name="esc", tag="esc")
                rhs = gpool3.tile([P, Bg * COLS_OUT], bf16, name="rhs",
                                  tag="rhs")
                r3 = rhs[:].rearrange("p (b e) -> p b e", e=COLS_OUT)
                if "exp" not in omit:
                    nc.scalar.activation(ex1[:], sc[:], AF.Exp)
                    nc.scalar.activation(ex2[:], sc[:], AF.Exp, scale=NEG)
                    nc.vector.tensor_tensor(
                        out=esc[:], in0=ex1[:], in1=ex2[:], op=OP.max)
                    nc.vector.tensor_copy(
                        r3[:, :, D:COLS_OUT],
                        esc[:].rearrange("p (b e) -> p b e", e=8))
                gf8 = g[:].bitcast(fp8)
                feat4 = bass.AP(gf8.tensor, gf8.offset,
                                [list(gf8.ap[0]), [GROWB, Bg],
                                 [HEADS, HID], [1, HEADS]])
                esc4 = bass.AP(esc[:].tensor, esc[:].offset,
                               [list(esc[:].ap[0]), [8, Bg], [0, HID],
                                [1, HEADS]])
                out4 = bass.AP(rhs[:].tensor, rhs[:].offset,
                               [list(rhs[:].ap[0]), [COLS_OUT, Bg],
                                [HEADS, HID], [1, HEADS]])
                if "mul" not in omit:
                    nc.vector.tensor_tensor(
                        out=out4, in0=feat4, in1=esc4, op=OP.mult)
                return rhs

            def window_compute(l, s, rhs, offg):
                B = int(cpw[s])
                off = int(offs[s])
                lo = off - offg

                S = gpool.tile([P, B * P], bf16, name="S", tag="S")
                if "s" not in omit:
                    for b in range(B):
                        nc.vector.tensor_scalar(
                            out=S[:, b * P:(b + 1) * P], in0=iota_s[:],
                            scalar1=dstl_s[:, off + b:off + b + 1],
                            scalar2=None, op0=OP.is_equal)

                pw = ps_pw.tile([P, COLS_OUT], f32, name="pw", tag="pw",
                                space="PSUM")
                if "mm" not in omit:
                    for b in range(B):
                        nc.tensor.matmul(
                            pw[:], lhsT=S[:, b * P:(b + 1) * P],
                            rhs=rhs[:, (lo + b) * COLS_OUT:(lo + b + 1) * COLS_OUT],
                            start=(b == 0), stop=(b == B - 1))
                else:
                    nc.vector.memset(pw[:], 0.0)
                return pw

            def table2_window(j):
                stg = stg2[j % 2][:, 0:ROWB]
                pfeat = ps_scr.tile([P, D], f32, name="pfeat2", tag="scr",
                                    space="PSUM")
                nc.tensor.matmul(pfeat[:], lhsT=h1T0[:, j * P:(j + 1) * P],
                                 rhs=W2b_s[:, 0:D], start=True, stop=False)
                nc.tensor.matmul(pfeat[:], lhsT=h1T1[:, j * P:(j + 1) * P],
                                 rhs=W2b_s[:, D:2 * D], start=False,
                                 stop=True)
                stg_f8 = stg[:].bitcast(fp8)
                nc.vector.tensor_copy(stg_f8[:, 0:D], pfeat[:])
                stg_bf = stg[:].bitcast(bf16)
                e2 = elr2_st[:, j * 16:(j + 1) * 16]
                nc.scalar.activation(stg_bf[:, 128:136], e2[:, 0:8], AF.Copy)
                er_v = er_all[:].bitcast(bf16)
                nc.vector.tensor_copy(
                    er_v[:, j * (ERB // 2):j * (ERB // 2) + 8], e2[:, 8:16])
                nc.sync.dma_start(cc_in1[j * P:(j + 1) * P, :], stg[:])

            def normalize(t_merged, l, j, pool_acc):
                """t_merged: [P, 264] f32 sbuf tile -> epilogue for window j."""
                dmx = wp.tile([P, 8], f32, name="dmx", tag="dmx")
                nc.vector.tensor_scalar_max(dmx[:], t_merged[:, D:COLS_OUT],
                                            1e-30)
                rec = wp.tile([P, 8], f32, name="rec", tag="rec")
                nc.vector.reciprocal(rec[:], dmx[:])
                rec_b = bass.AP(rec[:].tensor, rec[:].offset,
                                [list(rec[:].ap[0]), [0, HID], [1, HEADS]])
                hv = wp.tile([P, D], f32, name="hv", tag="hv")
                nc.vector.tensor_tensor(
                    out=hv[:].rearrange("p (d e) -> p d e", d=HID),
                    in0=t_merged[:, 0:D].rearrange("p (d e) -> p d e", d=HID),
                    in1=rec_b, op=OP.mult)
                bias = b1_s if l == 0 else b2_s
                nc.vector.tensor_tensor(out=hv[:], in0=hv[:], in1=bias[:],
                                        op=OP.add)
                if l == 0:
                    h1w = wp.tile([P, D], f32, name="h1w", tag="h1w")
                    nc.scalar.activation(h1w[:], hv[:], AF.Relu)
                    for half in (0, 1):
                        pT = ps_scr.tile([P, P], f32, name="pT2", tag="scr",
                                         space="PSUM")
                        nc.tensor.transpose(
                            pT[:], h1w[:, half * P:(half + 1) * P], id_s[:])
                        dstT = h1T0 if half == 0 else h1T1
                        nc.vector.tensor_copy(dstT[:, j * P:(j + 1) * P],
                                              pT[:])
                    pelr2 = ps2.tile([P, 16], f32, name="pelr2b", tag="pelr",
                                     space="PSUM")
                    for half in (0, 1):
                        dstT = h1T0 if half == 0 else h1T1
                        nc.tensor.matmul(
                            pelr2[:], lhsT=dstT[:, j * P:(j + 1) * P],
                            rhs=Welr2b_s[:, half * 16:(half + 1) * 16],
                            start=(half == 0), stop=(half == 1))
                    nc.vector.tensor_copy(elr2_st[:, j * 16:(j + 1) * 16],
                                          pelr2[:])
                    table2_window(j)
                else:
                    h2b = wp.tile([P, D + 1], bf16, name="h2b", tag="h2b")
                    nc.scalar.activation(h2b[:, 0:D], hv[:], AF.Relu)
                    nc.vector.tensor_copy(h2b[:, D:D + 1], ones_s[:])
                    Gw = wp.tile([P, 64], bf16, name="Gw", tag="Gw")
                    nc.vector.tensor_tensor(
                        out=Gw[:], in0=gl_s[:, j:j + 1].to_broadcast([P, 64]),
                        in1=iota_s[:, 0:64], op=OP.is_equal)
                    ppj = ps_scr.tile([64, D + 1], f32, name="ppj", tag="scr",
                                      space="PSUM")
                    nc.tensor.matmul(ppj[:], lhsT=Gw[:], rhs=h2b[:],
                                     start=True, stop=True)
                    nc.vector.tensor_tensor(out=pool_acc[:], in0=pool_acc[:],
                                            in1=ppj[:], op=OP.add)

            def main_layer(l, er_bar, pool_acc):
                def phase(base, is_A):
                    groups = [(base + ga, base + min(ga + grp, WPH))
                              for ga in range(0, WPH, grp)]
                    n = len(groups)
                    gbuf = {}
                    rbuf = {}
                    for k in range(n + 2):
                        if k < n:
                            ga, gb_ = groups[k]
                            gbuf[k] = gather_group(l, ga, gb_, er_bar)
                        if 0 <= k - 1 < n:
                            g, e, offg, Bg = gbuf.pop(k - 1)
                            rbuf[k - 1] = (score_group(l, g, e, offg, Bg),
                                           offg)
                        if 0 <= k - 2 < n:
                            rhs_g, offg = rbuf.pop(k - 2)
                            ga, gb_ = groups[k - 2]
                            for sidx in range(ga, gb_):
                                pw = window_compute(l, sidx, rhs_g, offg)
                                i = sidx - base
                                if is_A:
                                    nc.scalar.activation(
                                        stash[:, i * COLS_OUT:(i + 1) * COLS_OUT],
                                        pw[:], AF.Copy)
                                else:
                                    tm = wp.tile([P, COLS_OUT], f32,
                                                 name="tm", tag="tm")
                                    nc.vector.tensor_tensor(
                                        out=tm[:], in0=pw[:],
                                        in1=stash[:, i * COLS_OUT:(i + 1) * COLS_OUT],
                                        op=OP.add)
                                    normalize(tm, l, i, pool_acc)
                                    if l == 0 and i == NWA - 1 and stage >= 3:
                                        table1_exchange_A()

                # phase A: partner-half windows -> stash -> pship
                phase(0, True)
                shipw = nc.gpsimd.indirect_dma_start(
                    out=hold["pship"][l][:, :],
                    out_offset=bass.IndirectOffsetOnAxis(
                        ap=offs_shw_s[:, 0:1], axis=0),
                    in_=stash[:], in_offset=None)
                if not skip_bar:
                    bar2 = pair_barrier([shipw], f"ship_bar{l}")
                pgat = nc.gpsimd.indirect_dma_start(
                    out=stash[:], out_offset=None,
                    in_=hold["pship"][l][:, :],
                    in_offset=bass.IndirectOffsetOnAxis(
                        ap=offs_shr_s[:, 0:1], axis=0))
                if not skip_bar:
                    add_dep_helper(_raw(pgat), _raw(bar2), sync=True,
                                   reason="pship")
                # phase B: own-half windows, merge with partner partials
                phase(WPH, False)

            # =========================================================
            # program
            # =========================================================
            def emit_stub_out():
                z = pp.tile([64, NCLS], f32, name="zstub")
                nc.vector.memset(z[:], 0.0)
                nc.scalar.dma_start(out_t[:, :], z[:])

            pool_acc = pp.tile([64, D + 1], f32, name="pool_acc")
            nc.vector.memset(pool_acc[:], 0.0)
            if stage >= 1:
                for _rep in range(repeat):
                    hold["er_tab1"] = er_tab1_all[_rep]
                    hold["pship"] = pship_all[_rep]
                    table0_build()
                    if stage >= 2:
                        main_layer(0, None, None)
                    if stage >= 3:
                        bar_t2 = table1_exchange()
                    if stage >= 4:
                        main_layer(1, bar_t2, pool_acc)
            if stage < 4:
                emit_stub_out()

            if stage >= 4:
                # ---------- pooling AllReduce + MLP ----------
                nc.scalar.dma_start(ar_in[:, :], pool_acc[:])
                if not skip_ag and num_cores > 1:
                    nc.gpsimd.collective_compute(
                        "AllReduce", OP.add,
                        replica_groups=[list(range(num_cores))],
                        ins=[ar_in[:].opt()], outs=[ar_out[:].opt()])
                elif num_cores == 1:
                    nc.scalar.dma_start(ar_out[:, :], ar_in[:, :])
                hg_all = pp.tile([64, D + 1], f32, name="hg_all")
                nc.scalar.dma_start(
                    hg_all[:],
                    (ar_out if not skip_ag or num_cores == 1 else ar_in)[:, :])

                cmax = pp.tile([64, 1], f32, name="cmax")
                nc.vector.tensor_scalar_max(cmax[:], hg_all[:, D:D + 1], 1.0)
                crec = pp.tile([64, 1], f32, name="crec")
                nc.vector.reciprocal(crec[:], cmax[:])
                hg = pp.tile([64, D], f32, name="hg")
                nc.vector.tensor_tensor(out=hg[:], in0=hg_all[:, 0:D],
                                        in1=crec[:].to_broadcast([64, D]),
                                        op=OP.mult)

                Wc1_s = cp.tile([P, 64], f32, name="Wc1_s")
                nc.scalar.dma_start(Wc1_s[:, 0:32], Wc1[0:P, :])
                nc.scalar.dma_start(Wc1_s[:, 32:64], Wc1[P:2 * P, :])
                Wc2_s = load(Wc2, [32, 32], f32, "Wc2_s")
                Wc3_s = load(Wc3, [32, NCLS], f32, "Wc3_s")
                bc1_s = load(bc1rep, [64, 32], f32, "bc1_s")
                bc2_s = load(bc2rep, [64, 32], f32, "bc2_s")
                bc3_s = load(bc3rep, [64, NCLS], f32, "bc3_s")

                hgT = []
                for half in (0, 1):
                    pT = ps_scr.tile([P, 64], f32, name="pTm", tag="scr",
                                     space="PSUM")
                    nc.tensor.transpose(pT[:], hg[:, half * P:(half + 1) * P],
                                        id_s[0:64, 0:64])
                    t = pp.tile([P, 64], f32, name=f"hgT{half}")
                    nc.vector.tensor_copy(t[:], pT[:])
                    hgT.append(t)
                po1 = ps_scr.tile([64, 32], f32, name="po1", tag="scr",
                                  space="PSUM")
                nc.tensor.matmul(po1[:], lhsT=hgT[0][:], rhs=Wc1_s[:, 0:32],
                                 start=True, stop=False)
                nc.tensor.matmul(po1[:], lhsT=hgT[1][:], rhs=Wc1_s[:, 32:64],
                                 start=False, stop=True)
                o1 = pp.tile([64, 32], f32, name="o1")
                nc.vector.tensor_tensor(out=o1[:], in0=po1[:], in1=bc1_s[:],
                                        op=OP.add)
                nc.scalar.activation(o1[:], o1[:], AF.Relu)
                pT1 = ps_scr.tile([32, 64], f32, name="pT1", tag="scr",
                                  space="PSUM")
                nc.tensor.transpose(pT1[:], o1[:], id_s[0:64, 0:64])
                o1T = pp.tile([32, 64], f32, name="o1T")
                nc.vector.tensor_copy(o1T[:], pT1[:])
                po2 = ps_scr.tile([64, 32], f32, name="po2", tag="scr",
                                  space="PSUM")
                nc.tensor.matmul(po2[:], lhsT=o1T[:], rhs=Wc2_s[:],
                                 start=True, stop=True)
                o2 = pp.tile([64, 32], f32, name="o2")
                nc.vector.tensor_tensor(out=o2[:], in0=po2[:], in1=bc2_s[:],
                                        op=OP.add)
                nc.scalar.activation(o2[:], o2[:], AF.Relu)
                pT2m = ps_scr.tile([32, 64], f32, name="pT2m", tag="scr",
                                   space="PSUM")
                nc.tensor.transpose(pT2m[:], o2[:], id_s[0:64, 0:64])
                o2T = pp.tile([32, 64], f32, name="o2T")
                nc.vector.tensor_copy(o2T[:], pT2m[:])
                po3 = ps_scr.tile([64, NCLS], f32, name="po3", tag="scr",
                                  space="PSUM")
                nc.tensor.matmul(po3[:], lhsT=o2T[:], rhs=Wc3_s[:],
                                 start=True, stop=True)
                o3 = pp.tile([64, NCLS], f32, name="o3")
                nc.vector.tensor_tensor(out=o3[:], in0=po3[:], in1=bc3_s[:],
                                        op=OP.add)
                nc.scalar.dma_start(out_t[:, :], o3[:])

    nc.compile()
    return nc


def kernel(**inputs) -> np.ndarray:
    cfg = FULL
    in_maps, cpw, offs = host_prep(cfg, **inputs)
    nc = build_program(cfg, cpw, offs, num_cores=8)
    from concourse import bass_utils
    r = bass_utils.run_bass_kernel_spmd(nc, in_maps, core_ids=list(range(8)))
    return r.results[0]["out"]
